# revision 48
# baseline (speedup 1.0000x reference)
"""Additive (Bahdanau-style) attention kernel for Trainium2, 8 NeuronCores.

Reference computation (B=64, S=2048, D=512, H=512, f32):
    di    = hidden @ Wd.T + bd                      # [B, H]
    ei    = context @ We.T + be                     # [B, S, H]
    ui    = einsum('h,bsh->bs', V, tanh(di + ei))   # [B, S]
    ui    = where(mask, -inf, ui)
    alpha = softmax(ui, axis=-1)                    # [B, S]
    ah    = einsum('bsh,bs->bh', ei, alpha)         # [B, H]
    return alpha, ah

Strategy: data-parallel over batch (8 batches/core). Host stages
context transposed per batch (ctxT[b] = context[b].T, [D, S]) so the
contraction dim d sits on SBUF partitions; projection weights are
staged transposed + replicated. On-device per (batch, s-chunk of 512):
  zT[h,s] = sum_d WeT-block.T @ ctxT-chunk          (PE, f32r)
  t = tanh(zT + (di+be)[h])                          (ACT, per-partition bias)
  ui[s] += V-chunk.T @ t                             (PE, M=1, PSUM-accum)
  ui    += 1 * (-1e5*mask)                           (PE, K=1 -> exp underflows to 0)
  e = exp(ui) (+ accum denominator)                  (ACT)
  abcast = broadcast e-chunk to 128 partitions       (GPSIMD or PE ones-MM)
  parts[dc, sc] = sum_s ctxT[dc,s]*abcast            (DVE scalar_tensor_tensor)
Batch epilogue uses the identity  ah = (a~ . context) @ We.T / denom + be:
  acc[dc] = sum_sc parts  -> f32r via ACT copy
  psum_ah = sum_dc acc-chunk.T @ WeT-chunk           (PE, M=1)
  ah = psum_ah * (1/denom) + be                      (DVE)
alpha = e * (1/denom) per batch (DVE), DMA'd out per batch.

float32r notes: walrus requires each producer feeding an FP32r matmul
to emit float32r itself. Proven-on-HW producers: DMA (dram f32r ->
sbuf f32r) and ACT writes. DVE tensor_tensor_reduce crashes at
runtime, so the weighted-sum uses scalar_tensor_tensor partials
accumulated into f32 and converted via an ACT copy.
"""

import os as _os

import numpy as np

import concourse.bass as bass
import concourse.bacc as bacc
import concourse.mybir as mybir
import concourse.tile as tile
from concourse.bass_utils import run_bass_kernel_spmd

B, S, D, H = 64, 2048, 512, 512
NCORES = 8
BPC = B // NCORES          # batches per core
P = 128                    # SBUF partitions
DC = D // P                # 4 d-chunks
HC = H // P                # 4 h-chunks
SCHUNK = 512
SC = S // SCHUNK           # 4 s-chunks
NEG_BIG = -1.0e5           # folded into ui at masked positions; exp() -> 0

F32 = mybir.dt.float32
F32R = mybir.dt.float32r
BF16 = mybir.dt.bfloat16

MM_MODE = _os.environ.get("MM_MODE", "f32r")   # "f32r" | "bf16"
MMDT = BF16 if MM_MODE == "bf16" else F32R
# e-broadcast path: "gpsimd" partition_broadcast or "pe" ones-matmul
BCAST = _os.environ.get("BCAST", "gpsimd")
# weighted-sum partial op: "stt" (fused mul+reduce) or "tt" (2 plain ops)
ACCOP = _os.environ.get("ACCOP", "stt")
# denominator: "act" (exp accum_out) or "dve" (reduce over e row)
SIGMA = _os.environ.get("SIGMA", "act")
CTXBUFS = int(_os.environ.get("CTXBUFS", "2"))
PSZBUFS = int(_os.environ.get("PSZBUFS", "3"))
PSUIBUFS = int(_os.environ.get("PSUIBUFS", "2"))
TBUFS = int(_os.environ.get("TBUFS", "4"))
BCBUFS = int(_os.environ.get("BCBUFS", "2"))
EPIPOS = int(_os.environ.get("EPIPOS", "2"))


def _v(ap):
    """View an f32r AP as f32 for DVE/DMA consumers (free bitcast)."""
    if ap.dtype == F32R:
        return ap.bitcast(F32)
    return ap


def build_program():
    nc = bacc.Bacc(None, target_bir_lowering=False)

    # ---- per-core DRAM parameters (names = in_map keys) ----
    ctxT = nc.declare_dram_parameter("ctxT", [BPC, D, S], MMDT, isOutput=False)
    hidT = nc.declare_dram_parameter("hidT", [P, DC * BPC], MMDT, isOutput=False)
    maskneg = nc.declare_dram_parameter("maskneg", [BPC, S], MMDT, isOutput=False)
    weT = nc.declare_dram_parameter("weT", [D, H], MMDT, isOutput=False)
    wdT = nc.declare_dram_parameter("wdT", [D, H], MMDT, isOutput=False)
    # packed small constants: pack2 = [bdbe | vcols] per-partition,
    # pack1 = [be | ones] single-row (f32/f32r share 4-byte bits)
    pack2 = nc.declare_dram_parameter("pack2", [P, DC + HC], MMDT, isOutput=False)
    pack1 = nc.declare_dram_parameter("pack1", [1, H + P + 1], MMDT, isOutput=False)
    alpha_out = nc.declare_dram_parameter("alpha", [BPC, S], F32, isOutput=True)
    ah_out = nc.declare_dram_parameter("ah", [1, BPC * H], F32, isOutput=True)

    with tile.TileContext(nc) as tc:
        with (
            tc.tile_pool(name="const", bufs=1) as cpool,
            tc.tile_pool(name="ctx", bufs=CTXBUFS) as ctxpool,
            tc.tile_pool(name="work", bufs=3) as wpool,
            tc.tile_pool(name="psz", bufs=PSZBUFS, space="PSUM") as psz,
            tc.tile_pool(name="psui", bufs=PSUIBUFS, space="PSUM") as psui,
        ):
            # ---- one-time loads (diT inputs first: they gate the first
            # PE work; weT + batch-0's first columns next) ----
            hidT_sb = cpool.tile([P, DC * BPC], MMDT, tag="hidT")
            nc.sync.dma_start(hidT_sb[:], hidT[:])
            wdT_sb = []
            for dc in range(DC):
                w2 = cpool.tile([P, H], MMDT, tag=f"wdT{dc}")
                nc.sync.dma_start(w2[:], wdT[dc * P:(dc + 1) * P, :])
                wdT_sb.append(w2)
            weT_sb = []
            for dc in range(DC):
                w = cpool.tile([P, H], MMDT, tag=f"weT{dc}")
                nc.sync.dma_start(w[:], weT[dc * P:(dc + 1) * P, :])
                weT_sb.append(w)
            pack2_sb = cpool.tile([P, DC + HC], MMDT, tag="pack2")
            nc.sync.dma_start(pack2_sb[:], pack2[:])
            pack1_sb = cpool.tile([1, H + P + 1], MMDT, tag="pack1")
            nc.sync.dma_start(pack1_sb[:], pack1[:])
            bdbe_sb = _v(pack2_sb[:, 0:DC])
            vcols_sb = pack2_sb[:, DC:DC + HC]
            be_sb = _v(pack1_sb[:, 0:H])
            ones_sb = pack1_sb[:, H:H + P + 1]

            # persistent accumulators / stats (parity-duplicated where the
            # pipelined epilogue of batch b-1 overlaps batch b's compute)
            den = cpool.tile([1, BPC], F32, tag="den")
            rec = cpool.tile([1, BPC], F32, tag="rec")
            ah_all = cpool.tile([1, BPC * H], F32, tag="ah_all")
            biasT = cpool.tile([P, HC * BPC], F32, tag="biasT")
            sig2 = [
                cpool.tile([1, SC], F32, tag=f"sig{p}", name=f"sig{p}")
                for p in range(2)
            ]
            parts2 = [
                cpool.tile([P, DC * SC], F32, tag=f"parts{p}", name=f"parts{p}")
                for p in range(2)
            ]
            accf2 = [
                cpool.tile([P, DC], F32, tag=f"accf{p}", name=f"accf{p}")
                for p in range(2)
            ]
            accr2 = [
                cpool.tile([P, DC], F32R if MM_MODE == "f32r" else BF16,
                           tag=f"accr{p}", name=f"accr{p}")
                for p in range(2)
            ]

            # ---- diT = (Wd @ hidden.T) chunks; biasT[h, b] = diT + bd + be ----
            for hc in range(HC):
                ps = psz.tile([P, BPC], F32, tag="z")
                for dc in range(DC):
                    nc.tensor.matmul(
                        ps[:],
                        wdT_sb[dc][:, hc * P:(hc + 1) * P],
                        hidT_sb[:, dc * BPC:(dc + 1) * BPC],
                        start=(dc == 0),
                        stop=(dc == DC - 1),
                    )
                nc.vector.tensor_scalar_add(
                    biasT[:, hc * BPC:(hc + 1) * BPC], ps[:], bdbe_sb[:, hc:hc + 1]
                )

            # ---- main loop over this core's batches ----
            # The epilogue of batch b-1 (denominator, ah projection, alpha
            # scale + DMA) is emitted after batch b's first s-chunk so its
            # serial ACT/DVE chain doesn't stall batch b's first tanh.
            pend = {}

            def epilogue(b):
                pt = b % 2
                e_p = pend[b]
                nc.vector.reduce_sum(
                    den[:, b:b + 1], sig2[pt][:], axis=mybir.AxisListType.X
                )
                nc.vector.reciprocal(rec[:, b:b + 1], den[:, b:b + 1])
                ps_ah = psui.tile([1, H], F32, tag="ui", name="ps_ah")
                for dc in range(DC):
                    nc.vector.reduce_sum(
                        accf2[pt][:, dc:dc + 1],
                        parts2[pt][:, dc * SC:(dc + 1) * SC],
                        axis=mybir.AxisListType.X,
                    )
                    nc.scalar.activation(
                        accr2[pt][:, dc:dc + 1],
                        accf2[pt][:, dc:dc + 1],
                        mybir.ActivationFunctionType.Copy,
                    )
                    nc.tensor.matmul(
                        ps_ah[:],
                        accr2[pt][:, dc:dc + 1],
                        weT_sb[dc][:],
                        start=(dc == 0),
                        stop=(dc == DC - 1),
                    )
                # ah = ps_ah * (1/den) + be in one DVE op
                nc.vector.scalar_tensor_tensor(
                    out=ah_all[:, b * H:(b + 1) * H],
                    in0=ps_ah[:],
                    scalar=rec[:, b:b + 1],
                    in1=be_sb,
                    op0=mybir.AluOpType.mult,
                    op1=mybir.AluOpType.add,
                )
                # alpha = e * (1/den)
                nc.vector.tensor_scalar_mul(
                    _v(e_p[:]), _v(e_p[:]), rec[:, b:b + 1]
                )
                nc.sync.dma_start(alpha_out[b:b + 1, :], _v(e_p[:]))

            for b in range(BPC):
                pt = b % 2
                ctx_sb = []
                for dc in range(DC):
                    c = ctxpool.tile([P, S], MMDT, tag=f"ctx{dc}")
                    ctx_sb.append(c)
                load_scs = range(SC)
                # column-split loads: the first s-chunk's operands land ~4x
                # sooner than one [P, S] transfer per d-chunk
                for sc in load_scs:
                    s0 = sc * SCHUNK
                    for dc in range(DC):
                        nc.sync.dma_start(
                            ctx_sb[dc][:, s0:s0 + SCHUNK],
                            ctxT[b, dc * P:(dc + 1) * P, s0:s0 + SCHUNK],
                        )
                mneg_b = wpool.tile([1, S], MMDT, tag="mneg", bufs=2)
                nc.sync.dma_start(mneg_b[:], maskneg[b:b + 1, :])
                # e_b feeds a PE matmul only on the "pe" broadcast path
                ebdt = MMDT if BCAST == "pe" else F32
                e_b = wpool.tile([1, S], ebdt, tag="e_b", bufs=2)
                pend[b] = e_b

                for sch in range(SC // 2):
                  t_half = [None, None]
                  for hc in range(HC):
                    g0 = sch * 2 * SCHUNK
                    ps_z = psz.tile([P, 2 * SCHUNK], F32, tag="z")
                    for half in range(2):
                        hof = half * SCHUNK
                        for dc in range(DC):
                            nc.tensor.matmul(
                                ps_z[:, hof:hof + SCHUNK],
                                weT_sb[dc][:, hc * P:(hc + 1) * P],
                                ctx_sb[dc][:, g0 + hof:g0 + hof + SCHUNK],
                                start=(dc == 0),
                                stop=(dc == DC - 1),
                                skip_group_check=True,
                            )
                    t = wpool.tile([P, 2 * SCHUNK], MMDT, tag="t", bufs=TBUFS)
                    nc.scalar.activation(
                        t[:],
                        ps_z[:],
                        mybir.ActivationFunctionType.Tanh,
                        bias=biasT[:, hc * BPC + b:hc * BPC + b + 1],
                    )
                    if hc == 0:
                        t_half = [t]
                    else:
                        t_half.append(t)
                  for half in range(2):
                    sc = sch * 2 + half
                    s0 = sc * SCHUNK
                    hof = half * SCHUNK
                    ps_ui = psui.tile([1, SCHUNK], F32, tag="ui")
                    for hc in range(HC):
                        nc.tensor.matmul(
                            ps_ui[:],
                            vcols_sb[:, hc:hc + 1],
                            t_half[hc][:, hof:hof + SCHUNK],
                            start=(hc == 0),
                            stop=False,
                            skip_group_check=True,
                        )
                    # masked positions: ui += 1 * (-1e5 * mask)
                    nc.tensor.matmul(
                        ps_ui[:],
                        ones_sb[:, P:P + 1],
                        mneg_b[:, s0:s0 + SCHUNK],
                        start=False,
                        stop=True,
                        skip_group_check=True,
                    )
                    # e = exp(ui); optionally sigma = sum_s e via accum
                    if SIGMA == "act":
                        nc.scalar.activation(
                            e_b[:, s0:s0 + SCHUNK],
                            ps_ui[:],
                            mybir.ActivationFunctionType.Exp,
                            accum_out=sig2[pt][:, sc:sc + 1],
                        )
                    else:
                        nc.scalar.activation(
                            e_b[:, s0:s0 + SCHUNK],
                            ps_ui[:],
                            mybir.ActivationFunctionType.Exp,
                        )
                    # broadcast e-chunk to 128 partitions
                    if BCAST == "gpsimd":
                        bc = wpool.tile([P, SCHUNK], F32, tag="bc", bufs=BCBUFS)
                        nc.gpsimd.partition_broadcast(
                            bc[:], _v(e_b[:, s0:s0 + SCHUNK])
                        )
                        bc_ap = bc[:]
                    else:
                        ps_bc = psz.tile([P, SCHUNK], F32, tag="bcp")
                        nc.tensor.matmul(
                            ps_bc[:],
                            ones_sb[:, 0:P],
                            e_b[:, s0:s0 + SCHUNK],
                            start=True,
                            stop=True,
                        )
                        bc_ap = ps_bc[:]
                    # parts[:, dc*SC+sc] = sum_s ctxT[dc-chunk, s] * e[s]
                    for dc in range(DC):
                        scr = wpool.tile([P, SCHUNK], F32, tag="ttr_scratch")
                        if ACCOP == "stt":
                            nc.vector.scalar_tensor_tensor(
                                out=scr[:],
                                in0=_v(ctx_sb[dc][:, s0:s0 + SCHUNK]),
                                scalar=1.0,
                                in1=bc_ap,
                                op0=mybir.AluOpType.mult,
                                op1=mybir.AluOpType.mult,
                                accum_out=parts2[pt][:, dc * SC + sc:dc * SC + sc + 1],
                            )
                        else:
                            nc.vector.tensor_mul(
                                scr[:], _v(ctx_sb[dc][:, s0:s0 + SCHUNK]), bc_ap
                            )
                            nc.vector.reduce_sum(
                                parts2[pt][:, dc * SC + sc:dc * SC + sc + 1],
                                scr[:],
                                axis=mybir.AxisListType.X,
                            )
                    if sc == EPIPOS and b > 0:
                        epilogue(b - 1)

            epilogue(BPC - 1)

            nc.sync.dma_start(ah_out[:], ah_all[:])

    nc.compile()
    return nc


_CACHED = {}


def _get_program():
    if "nc" not in _CACHED:
        _CACHED["nc"] = build_program()
    return _CACHED["nc"]


def make_in_maps(hidden, context, mask, Wd, bd, We, be, V):
    hidden = np.asarray(hidden, dtype=np.float32)
    context = np.asarray(context, dtype=np.float32)
    mask = np.asarray(mask)
    Wd = np.asarray(Wd, dtype=np.float32)
    bd = np.asarray(bd, dtype=np.float32)
    We = np.asarray(We, dtype=np.float32)
    be = np.asarray(be, dtype=np.float32)
    V = np.asarray(V, dtype=np.float32)

    if MM_MODE == "bf16":
        import ml_dtypes
        mmnp = ml_dtypes.bfloat16
    else:
        mmnp = np.float32

    weT_np = np.ascontiguousarray(We.T).astype(mmnp)          # [D, H]
    wdT_np = np.ascontiguousarray(Wd.T).astype(mmnp)          # [D, H]
    bdbe_np = np.ascontiguousarray((bd + be).reshape(DC, P).T)  # [P, DC]
    vcols_np = np.ascontiguousarray(V.reshape(HC, P).T)          # [P, HC]
    pack2_np = np.concatenate([bdbe_np, vcols_np], axis=1).astype(mmnp)
    pack1_np = np.concatenate(
        [be.reshape(1, H), np.ones((1, P + 1), dtype=np.float32)], axis=1
    ).astype(mmnp)

    in_maps = []
    for c in range(NCORES):
        b0 = c * BPC
        ctx_c = context[b0:b0 + BPC]                          # [BPC, S, D]
        ctxT_c = np.ascontiguousarray(ctx_c.transpose(0, 2, 1)).astype(mmnp)
        hid_t = hidden[b0:b0 + BPC].T                      # [D, BPC]
        hidT_c = np.ascontiguousarray(
            hid_t.reshape(DC, P, BPC).transpose(1, 0, 2).reshape(P, DC * BPC)
        ).astype(mmnp)
        maskneg_c = (NEG_BIG * mask[b0:b0 + BPC].astype(np.float32)).astype(mmnp)
        in_maps.append({
            "ctxT": ctxT_c,
            "hidT": hidT_c,
            "maskneg": np.ascontiguousarray(maskneg_c),
            "weT": weT_np,
            "wdT": wdT_np,
            "pack2": pack2_np,
            "pack1": pack1_np,
        })
    return in_maps


def kernel(hidden, context, mask, Wd, bd, We, be, V):
    in_maps = make_in_maps(hidden, context, mask, Wd, bd, We, be, V)
    nc = _get_program()
    res = run_bass_kernel_spmd(nc, in_maps, list(range(NCORES)))
    outs = res.results

    alpha = np.concatenate(
        [outs[c]["alpha"] for c in range(NCORES)], axis=0
    )
    ah = np.concatenate(
        [outs[c]["ah"].reshape(BPC, H) for c in range(NCORES)], axis=0
    )
    return alpha.astype(np.float32), ah.astype(np.float32)
